# revision 28
# baseline (speedup 1.0000x reference)
"""Trainium2 Bass kernel for nn_Conv4d (K separate Conv3d layers folded into a
single conv3d with K*Co output channels + temporal accumulation).

Problem (hardcoded):
  x:      [B=2, Ci=8, T=16, D=40, H=40, W=40] f32
  weight: [K=3, Co=32, Ci=8, 3, 3, 3] f32
  bias:   [K=3, Co=32] f32
  out:    [B=2, Co=32, O=16, 40, 40, 40] f32
  frame j contributes through filter k to output frame o = j + 1 - k
  (OOB temporal frames skipped, incl. their bias).

Sharding: data-parallel over the 32 B*T frames, 4 consecutive frames per core.
Partial sums for the overlapping output frames are combined on the host.

v3: the full 216-element contraction (kw3 x kh3 x kd3 x ci8) is packed into
partition rows and split into 2 matmul passes (128 + 88 rows + ones), so each
psum tile needs 2 matmuls per (j, h-chunk) instead of 3 -- 1280 total vs 1920.
The (kw, kh) window replication (9x) is prebuilt on the host per PLANE; the kd
dimension is handled by keeping a rolling window of 3 padded d-planes resident
in SBUF ("slots" = plane % 3) and rotating the WEIGHT rows per d (kd =
(slot - d) mod 3, 3 precomputed weight variants), so each plane's replicated
form is DMA'd once -- input HBM stays ~40 MB/core.

Device layout per core:
  - XA [128, 4j*1680] / XB [89, 4j*1680] bf16 persistent tiles; global row
    r = s*72 + kw*24 + kh*8 + ci (s = plane%3) holds, for each frame j, the
    contiguous 1680-elem flat window of padded plane (ci, t0+j, plane) starting
    at offset kh*42+kw. Rows 0..127 -> XA, 128..215 -> XB[0:88], XB[88] = ones.
    Per d, the slot of retiring plane d is overwritten with plane d+3 (first
    needed at d+1) in per-j chunks spread over the 3 DMA-issue queues.
  - psum tile A accumulates j=0,1 (2 passes x 2 j per h-chunk region), tile B
    j=2,3. M = 128 = 4 blocks of 32 channels; block layouts differ by j parity
    (even j: block mb holds k = 2-mb, block 3 zero; odd j: block mb holds
    k = 3-mb, block 0 zero) so psum block mb always holds output frame
    o = (t0-1 resp t0+1) + mb. Pass-2 includes the ones-row whose weight row
    is bias -> bias added once per (j,k) per output element.
  - evict psum -> bf16 SBUF stage (scalar engine for A, vector for B), then
    DMA stage -> DRAM outA/outB [128, D, 1600] bf16.
Host: out[o] = sum of A/B blocks mapping to o (fp32 adds of bf16 partials).
"""

import numpy as np

_STATE: dict = {}

# ---- problem constants --------------------------------------------------
B, CI, T, D, H, W = 2, 8, 16, 40, 40, 40
K, CO = 3, 32
O = 16
NCORES = 8
FRAMES = 4          # frames per core
DP, HP, WP = D + 2, H + 2, W + 2   # padded dims
HW = H * WP          # 40*42 = free size of one (h,w') window
NHC = 4              # h-chunks per d-slice
HCROWS = H // NHC    # 10 rows -> N=400 per matmul
R1 = 128             # pass-1 contraction rows
R2 = 89              # pass-2 rows (88 data + ones)


def _build_nc():
    import concourse.mybir as mybir
    from concourse import bacc
    from concourse.tile import TileContext

    f32 = mybir.dt.float32
    bf16 = mybir.dt.bfloat16

    nc = bacc.Bacc(
        "TRN2", target_bir_lowering=False, debug=False, num_devices=NCORES
    )
    # xslab[plane, row(kw kh ci), j*window]
    xs = nc.dram_tensor("xs", [DP, 72, FRAMES * HW], bf16, kind="ExternalInput")
    w1 = nc.dram_tensor("w1", [R1, 12 * 128], bf16, kind="ExternalInput")
    w2 = nc.dram_tensor("w2", [R2, 12 * 128], bf16, kind="ExternalInput")
    ones = nc.dram_tensor("ones", [1, FRAMES * HW], bf16, kind="ExternalInput")
    outA = nc.dram_tensor("outA", [128, D * H * W], bf16, kind="ExternalOutput")
    outB = nc.dram_tensor("outB", [64, D * H * W], bf16, kind="ExternalOutput")

    FHW = FRAMES * HW
    QS = None  # set below

    with TileContext(nc) as tc:
        with (
            tc.tile_pool(name="const", bufs=1) as pc,
            tc.tile_pool(name="st", bufs=4) as pst,
            tc.tile_pool(name="ps", bufs=2, space="PSUM") as pp,
        ):
            wt1 = pc.tile([R1, 12 * 128], bf16)
            wt2 = pc.tile([R2, 12 * 128], bf16)
            xa = pc.tile([R1, FHW], bf16)
            xb = pc.tile([R2, FHW], bf16)
            nc.sync.dma_start(wt1[0:64, :], w1[0:64, :])
            nc.sync.dma_start(wt1[64:128, :], w1[64:128, :])
            nc.gpsimd.dma_start(wt2[0:48, :], w2[0:48, :])
            nc.gpsimd.dma_start(wt2[48:89, :], w2[48:89, :])
            nc.scalar.dma_start(xb[88:89, :], ones[:, :])
            QS = [nc.sync, nc.gpsimd, nc.scalar]

            def write_chunk(plane, jc, qs=None):
                """DMA one j-chunk of plane's 72 replicated rows to its slot.

                Split into ~24-row pieces on alternating queues: each
                dma_start is one descriptor; per-DMA-engine bandwidth is only
                ~19 GB/s regardless of packet size, so throughput and arrival
                latency come from many outstanding descriptors fanning out
                across the 16 engines. Slot chunks' semaphore waits resolve
                late, so keep them off the sync queue (which carries the
                output DMAs) to avoid head-of-line blocking.
                """
                qs = qs or [nc.gpsimd, nc.scalar]
                s = plane % 3
                lo = jc * HW
                hi = lo + HW
                if s == 0:
                    pieces = [(xa, 0, 0, 36), (xa, 36, 36, 72)]
                elif s == 1:
                    pieces = [(xa, 72, 0, 28), (xa, 100, 28, 56), (xb, 0, 56, 72)]
                else:
                    pieces = [(xb, 16, 0, 36), (xb, 52, 36, 72)]
                for i, (tile, db, sa, sb) in enumerate(pieces):
                    qs[i % len(qs)].dma_start(
                        tile[db : db + sb - sa, lo:hi], xs[plane, sa:sb, lo:hi]
                    )

            # initial planes 0-2: j=0 chunks of all slots first (the first
            # matmuls need j=0 rows of every slot), spread over 3 queues
            for jc in range(FRAMES):
                for p in range(3):
                    r = p + jc
                    write_chunk(p, jc, [QS[r % 3], QS[(r + 1) % 3]])

            xav = xa[:, :].rearrange("p (j h w) -> p j h w", j=FRAMES, w=WP)
            xbv = xb[:, :].rearrange("p (j h w) -> p j h w", j=FRAMES, w=WP)
            for d in range(D):
                rot = d % 3
                for pair in range(2):
                    ps = pp.tile([128, NHC * 512], f32, tag="ps")
                    for jj in range(2):
                        j = pair * 2 + jj
                        g = rot * 4 + pair * 2 + jj
                        l1 = wt1[:, g * 128 : (g + 1) * 128]
                        l2 = wt2[:, g * 128 : (g + 1) * 128]
                        for hc in range(NHC):
                            out_ap = ps[:, hc * 512 : hc * 512 + HCROWS * W]
                            rhs1 = xav[
                                :, j, hc * HCROWS : (hc + 1) * HCROWS, 0:W
                            ]
                            rhs2 = xbv[
                                :, j, hc * HCROWS : (hc + 1) * HCROWS, 0:W
                            ]
                            nc.tensor.matmul(
                                out_ap, l1, rhs1,
                                start=(jj == 0), stop=False,
                            )
                            nc.tensor.matmul(
                                out_ap, l2, rhs2,
                                start=False, stop=(jj == 1),
                            )
                    psv = ps[:, :].rearrange("p (b c) -> p b c", c=512)[
                        :, :, 0 : HCROWS * W
                    ]
                    lo, hi = d * H * W, (d + 1) * H * W
                    if pair == 0:
                        stA = pst.tile([128, H * W], bf16, tag="stA")
                        sv = stA[:, :].rearrange("p (b c) -> p b c", c=HCROWS * W)
                        nc.scalar.copy(sv, psv)
                        nc.sync.dma_start(outA[0:64, lo:hi], stA[0:64, :])
                    else:
                        # pair B's weight blocks are swapped (+2 mod 4) so its
                        # overlap o-slices (t0+1, t0+2) land at partitions
                        # 64:128: merge them into stA on-chip, ship only
                        # B's unique half -- output traffic 32.8 -> 24.6 MB
                        stB = pst.tile([128, H * W], bf16, tag="stB")
                        sv = stB[:, :].rearrange("p (b c) -> p b c", c=HCROWS * W)
                        nc.vector.tensor_copy(sv, psv)
                        nc.vector.tensor_add(
                            stA[64:128, :], stB[64:128, :], stA[64:128, :]
                        )
                        nc.sync.dma_start(outA[64:128, lo:hi], stA[64:128, :])
                        nc.sync.dma_start(outB[0:64, lo:hi], stB[0:64, :])
                    # prefetch plane d+3: chunks j0/j1 right after pair A
                    # (their WAR deps -- readers of plane d, frames 0/1 --
                    # are already satisfied), j2/j3 after pair B
                    if d + 3 < DP:
                        for jc in (0, 1) if pair == 0 else (2, 3):
                            write_chunk(d + 3, jc)
    nc.compile()
    return nc


def _get_nc():
    if "nc" not in _STATE:
        _STATE["nc"] = _build_nc()
    return _STATE["nc"]


def _host_inputs(x, weight, bias):
    """Build per-core input maps."""
    import ml_dtypes

    bf16 = ml_dtypes.bfloat16
    x = np.asarray(x, dtype=np.float32)
    weight = np.ascontiguousarray(weight, dtype=np.float32)
    bias = np.ascontiguousarray(bias, dtype=np.float32)

    # weight [k, co, ci, kd, kh, kw] -> wrev [kh, kd, ci, kw, k'(=2-k), co]
    wrev = weight.transpose(4, 3, 2, 5, 0, 1)[:, :, :, :, ::-1, :]
    # Parity block layouts (col = mb*32 + co):
    #   par=0 (even j): blocks 0..2 = k reversed, block 3 zero
    #   par=1 (odd  j): block 0 zero, blocks 1..3 = k reversed
    w_par = np.zeros((2, 3, 3, 8, 3, 4, 32), np.float32)  # [par,kh,kd,ci,kw,mb,co]
    w_par[0, :, :, :, :, 0:3] = wrev
    w_par[1, :, :, :, :, 1:4] = wrev
    # Rotated row layouts: row r = s*72 + kw*24 + kh*8 + ci, kd = (s - rot)%3
    w1h = np.zeros((R1, 12, 128), np.float32)
    w2h = np.zeros((R2, 12, 128), np.float32)
    brev = bias[::-1].reshape(96)
    for rot in range(3):
        kd_of_s = [(s - rot) % 3 for s in range(3)]
        for par in range(2):
            # [kh, s, ci, kw, mb, co] -> (s, kw, kh, ci, mb*co)
            arr = w_par[par][:, kd_of_s]
            arr = arr.transpose(1, 3, 0, 2, 4, 5).reshape(216, 128)
            a1, a2 = arr[0:128], arr[128:216]
            b2 = np.zeros(128, np.float32)
            if par == 0:
                b2[0:96] = brev
            else:
                b2[32:128] = brev
            for pair in range(2):
                g = rot * 4 + pair * 2 + par
                if pair == 0:
                    w1h[:, g] = a1
                    w2h[0:88, g] = a2
                    w2h[88, g] = b2
                else:
                    # swap column-block halves: B's o-slices t0+1/t0+2
                    # move to partitions 64:128 of the psum
                    w1h[:, g] = np.concatenate([a1[:, 64:128], a1[:, 0:64]], 1)
                    w2h[0:88, g] = np.concatenate([a2[:, 64:128], a2[:, 0:64]], 1)
                    w2h[88, g] = np.concatenate([b2[64:128], b2[0:64]])
    w1h = np.ascontiguousarray(w1h.reshape(R1, 12 * 128)).astype(bf16)
    w2h = np.ascontiguousarray(w2h.reshape(R2, 12 * 128)).astype(bf16)
    onesh = np.ones((1, FRAMES * HW), bf16)

    xb16 = x.astype(bf16)
    PL = DP * HP * WP
    in_maps = []
    for c in range(NCORES):
        b, tb = divmod(c, 4)
        t0 = tb * FRAMES
        # padded frames with 8 elems of tail slack for the window overhang
        buf = np.zeros((FRAMES, CI, PL + 8), bf16)
        xpc = buf[:, :, :PL].reshape(FRAMES, CI, DP, HP, WP)
        xpc[:, :, 1 : 1 + D, 1 : 1 + H, 1 : 1 + W] = xb16[
            b, :, t0 : t0 + FRAMES
        ].transpose(1, 0, 2, 3, 4)
        js, cs, es = buf.strides
        ds, hs, ws = HP * WP * es, WP * es, es
        # slab[plane, (kw kh ci), j, f] = flat window @ kh*42+kw of plane
        win = np.lib.stride_tricks.as_strided(
            buf,
            shape=(DP, 3, 3, CI, FRAMES, HW),
            strides=(ds, ws, hs, cs, js, ws),
        )
        slab = np.ascontiguousarray(win).reshape(DP, 72, FRAMES * HW)
        in_maps.append(
            {"xs": slab, "w1": w1h, "w2": w2h, "ones": onesh}
        )
    return in_maps


def _assemble(results):
    out = np.zeros((B, CO, O, D, H, W), np.float32)
    for c in range(NCORES):
        b, tb = divmod(c, 4)
        t0 = tb * FRAMES
        A = results[c]["outA"].astype(np.float32).reshape(4, 32, D, H, W)
        Bv = results[c]["outB"].astype(np.float32).reshape(2, 32, D, H, W)
        for i in range(4):
            o = t0 - 1 + i
            if 0 <= o < O:
                out[b, :, o] += A[i]
        for i in range(2):
            o = t0 + 3 + i
            if 0 <= o < O:
                out[b, :, o] += Bv[i]
    return out


def _run(x, weight, bias, trace=False, tmpdir=None):
    from concourse.bass_utils import run_bass_kernel_spmd

    if trace:
        _install_ntff_hook()
    nc = _get_nc()
    in_maps = _host_inputs(x, weight, bias)
    res = run_bass_kernel_spmd(
        nc,
        in_maps,
        core_ids=list(range(NCORES)),
        trace=trace,
        tmpdir=tmpdir,
    )
    return _assemble(res.results), res.exec_time_ns


def _install_ntff_hook():
    """Register the axon NTFF profile hook (missing from this image's antenv)."""
    import sys, types

    if "antenv.axon_hooks" in sys.modules:
        return
    mod = types.ModuleType("antenv.axon_hooks")
    holder = [None]
    mod.set_axon_ntff_profile_hook = lambda h: holder.__setitem__(0, h)
    mod.get_axon_ntff_profile_hook = lambda: holder[0]
    sys.modules["antenv.axon_hooks"] = mod
    try:
        from trn_agent_boot.trn_boot import _ntff_profile_via_ctypes

        mod.set_axon_ntff_profile_hook(
            _ntff_profile_via_ctypes("/opt/axon/libaxon_pjrt.so")
        )
    except Exception:
        pass


def kernel(x, weight, bias):
    out, _ = _run(x, weight, bias, trace=False)
    return out


# revision 29
# speedup vs baseline: 1.0308x; 1.0308x over previous
"""Trainium2 Bass kernel for nn_Conv4d (K separate Conv3d layers folded into a
single conv3d with K*Co output channels + temporal accumulation).

Problem (hardcoded):
  x:      [B=2, Ci=8, T=16, D=40, H=40, W=40] f32
  weight: [K=3, Co=32, Ci=8, 3, 3, 3] f32
  bias:   [K=3, Co=32] f32
  out:    [B=2, Co=32, O=16, 40, 40, 40] f32
  frame j contributes through filter k to output frame o = j + 1 - k
  (OOB temporal frames skipped, incl. their bias).

Sharding: data-parallel over the 32 B*T frames, 4 consecutive frames per
core; the remaining cross-core output-frame overlap is summed on the host.

Design (~3.4x over the v1 baseline, ~300 us vs 1019 us):
  * bf16 data / fp32 psum. rel err ~3e-3 vs the 2e-2 gate.
  * Full 216-row contraction (kw3 x kh3 x kd3 x ci8) packed into partition
    rows, 2 matmul passes per psum region (128 + 88 rows + ones/bias row)
    instead of 3 kw-shifted passes: 1280 matmuls of N=400, the tensor floor.
    Full-128-row passes also keep the PE HAM clock-gate at K=8/8 (2.4 GHz,
    ~167 ns/matmul); narrow-contraction versions of this kernel never left
    the cold 1.2 GHz state.
  * The (kw, kh) window replication (9x) is prebuilt on the host per PLANE;
    kd is realized by keeping a rolling window of 3 padded d-planes resident
    in SBUF (slot = plane % 3 -> row range) and rotating the WEIGHT rows per
    d (kd = (slot - d) mod 3; 3 precomputed weight variants), so each
    plane's replicated form is DMA'd once: input HBM ~41 MB/core.
    Slot writes are split per-frame (j0/j1 after psum-pair A whose matmuls
    were their WAR dependency, j2/j3 after pair B) in ~36-row pieces
    alternated over the gpsimd/scalar issue queues so the transfers hide
    inside the next d-slice's compute.
  * psum tile A accumulates frames j=0,1 (2 passes x 2 j per 512-col
    h-chunk region), tile B j=2,3. M = 128 = 4 blocks of 32 channels;
    parity block layouts keep every eviction a partition-base-0 op.
  * Pair B's weight column blocks are swapped (+2 mod 4) so B's overlap
    o-slices (t0+1, t0+2) land on partitions 64:128; after evicting both
    pairs to bf16 SBUF stages (scalar engine for A, vector for B), a single
    aligned vector add merges B's overlap into stage A and only B's unique
    half ships: output HBM 24.6 MB/core instead of 32.8.
  * Output DMAs go as partition halves on the sync queue; issue queues are
    kept single-purpose (sync: outputs+wt1, gpsimd/scalar: slot pieces,
    alternating) -- a dma_start's semaphore wait blocks its whole queue, so
    mixing late-resolving slot writes with output DMAs stalls both.
Host: final fp32 output assembled from bf16 partials (A rows -> o = t0-1+i
fully merged for the in-core overlap; B rows -> o = t0+3+i; cross-core
boundary frames summed across cores).
"""

import numpy as np

_STATE: dict = {}

# ---- problem constants --------------------------------------------------
B, CI, T, D, H, W = 2, 8, 16, 40, 40, 40
K, CO = 3, 32
O = 16
NCORES = 8
FRAMES = 4          # frames per core
DP, HP, WP = D + 2, H + 2, W + 2   # padded dims
HW = H * WP          # 40*42 = free size of one (h,w') window
NHC = 4              # h-chunks per d-slice
HCROWS = H // NHC    # 10 rows -> N=400 per matmul
R1 = 128             # pass-1 contraction rows
R2 = 89              # pass-2 rows (88 data + ones)


def _build_nc():
    import concourse.mybir as mybir
    from concourse import bacc
    from concourse.tile import TileContext

    f32 = mybir.dt.float32
    bf16 = mybir.dt.bfloat16

    nc = bacc.Bacc(
        "TRN2", target_bir_lowering=False, debug=False, num_devices=NCORES
    )
    # xslab[plane, row(kw kh ci), j*window]
    xs = nc.dram_tensor("xs", [DP, 72, FRAMES * HW], bf16, kind="ExternalInput")
    w1 = nc.dram_tensor("w1", [R1, 12 * 128], bf16, kind="ExternalInput")
    w2 = nc.dram_tensor("w2", [R2, 12 * 128], bf16, kind="ExternalInput")
    ones = nc.dram_tensor("ones", [1, FRAMES * HW], bf16, kind="ExternalInput")
    outA = nc.dram_tensor("outA", [128, D * H * W], bf16, kind="ExternalOutput")
    outB = nc.dram_tensor("outB", [64, D * H * W], bf16, kind="ExternalOutput")

    FHW = FRAMES * HW
    QS = None  # set below

    with TileContext(nc) as tc:
        with (
            tc.tile_pool(name="const", bufs=1) as pc,
            tc.tile_pool(name="st", bufs=4) as pst,
            tc.tile_pool(name="ps", bufs=2, space="PSUM") as pp,
        ):
            wt1 = pc.tile([R1, 12 * 128], bf16)
            wt2 = pc.tile([R2, 12 * 128], bf16)
            xa = pc.tile([R1, FHW], bf16)
            xb = pc.tile([R2, FHW], bf16)
            nc.sync.dma_start(wt1[0:64, :], w1[0:64, :])
            nc.sync.dma_start(wt1[64:128, :], w1[64:128, :])
            nc.gpsimd.dma_start(wt2[0:48, :], w2[0:48, :])
            nc.gpsimd.dma_start(wt2[48:89, :], w2[48:89, :])
            nc.scalar.dma_start(xb[88:89, :], ones[:, :])
            QS = [nc.sync, nc.gpsimd, nc.scalar]

            def write_chunk(plane, jc, qs=None):
                """DMA one j-chunk of plane's 72 replicated rows to its slot.

                Split into ~24-row pieces on alternating queues: each
                dma_start is one descriptor; per-DMA-engine bandwidth is only
                ~19 GB/s regardless of packet size, so throughput and arrival
                latency come from many outstanding descriptors fanning out
                across the 16 engines. Slot chunks' semaphore waits resolve
                late, so keep them off the sync queue (which carries the
                output DMAs) to avoid head-of-line blocking.
                """
                qs = qs or [nc.gpsimd, nc.scalar]
                s = plane % 3
                lo = jc * HW
                hi = lo + HW
                if s == 0:
                    pieces = [(xa, 0, 0, 36), (xa, 36, 36, 72)]
                elif s == 1:
                    pieces = [(xa, 72, 0, 28), (xa, 100, 28, 56), (xb, 0, 56, 72)]
                else:
                    pieces = [(xb, 16, 0, 36), (xb, 52, 36, 72)]
                for i, (tile, db, sa, sb) in enumerate(pieces):
                    qs[i % len(qs)].dma_start(
                        tile[db : db + sb - sa, lo:hi], xs[plane, sa:sb, lo:hi]
                    )

            # initial planes 0-2: j=0 chunks of all slots first (the first
            # matmuls need j=0 rows of every slot), spread over 3 queues
            for jc in range(FRAMES):
                for p in range(3):
                    r = p + jc
                    write_chunk(p, jc, [QS[r % 3], QS[(r + 1) % 3]])

            xav = xa[:, :].rearrange("p (j h w) -> p j h w", j=FRAMES, w=WP)
            xbv = xb[:, :].rearrange("p (j h w) -> p j h w", j=FRAMES, w=WP)
            for d in range(D):
                rot = d % 3
                for pair in range(2):
                    ps = pp.tile([128, NHC * 512], f32, tag="ps")
                    for jj in range(2):
                        j = pair * 2 + jj
                        g = rot * 4 + pair * 2 + jj
                        l1 = wt1[:, g * 128 : (g + 1) * 128]
                        l2 = wt2[:, g * 128 : (g + 1) * 128]
                        for hc in range(NHC):
                            out_ap = ps[:, hc * 512 : hc * 512 + HCROWS * W]
                            rhs1 = xav[
                                :, j, hc * HCROWS : (hc + 1) * HCROWS, 0:W
                            ]
                            rhs2 = xbv[
                                :, j, hc * HCROWS : (hc + 1) * HCROWS, 0:W
                            ]
                            nc.tensor.matmul(
                                out_ap, l1, rhs1,
                                start=(jj == 0), stop=False,
                            )
                            nc.tensor.matmul(
                                out_ap, l2, rhs2,
                                start=False, stop=(jj == 1),
                            )
                    psv = ps[:, :].rearrange("p (b c) -> p b c", c=512)[
                        :, :, 0 : HCROWS * W
                    ]
                    lo, hi = d * H * W, (d + 1) * H * W
                    if pair == 0:
                        stA = pst.tile([128, H * W], bf16, tag="stA")
                        sv = stA[:, :].rearrange("p (b c) -> p b c", c=HCROWS * W)
                        nc.scalar.copy(sv, psv)
                        nc.sync.dma_start(outA[0:64, lo:hi], stA[0:64, :])
                    else:
                        # pair B's weight blocks are swapped (+2 mod 4) so its
                        # overlap o-slices (t0+1, t0+2) land at partitions
                        # 64:128: merge them into stA on-chip, ship only
                        # B's unique half -- output traffic 32.8 -> 24.6 MB
                        stB = pst.tile([128, H * W], bf16, tag="stB")
                        sv = stB[:, :].rearrange("p (b c) -> p b c", c=HCROWS * W)
                        nc.vector.tensor_copy(sv, psv)
                        nc.vector.tensor_add(
                            stA[64:128, :], stB[64:128, :], stA[64:128, :]
                        )
                        nc.sync.dma_start(outA[64:128, lo:hi], stA[64:128, :])
                        nc.sync.dma_start(outB[0:64, lo:hi], stB[0:64, :])
                    # prefetch plane d+3: chunks j0/j1 right after pair A
                    # (their WAR deps -- readers of plane d, frames 0/1 --
                    # are already satisfied), j2/j3 after pair B
                    if d + 3 < DP:
                        for jc in (0, 1) if pair == 0 else (2, 3):
                            write_chunk(d + 3, jc)
    nc.compile()
    return nc


def _get_nc():
    if "nc" not in _STATE:
        _STATE["nc"] = _build_nc()
    return _STATE["nc"]


def _host_inputs(x, weight, bias):
    """Build per-core input maps."""
    import ml_dtypes

    bf16 = ml_dtypes.bfloat16
    x = np.asarray(x, dtype=np.float32)
    weight = np.ascontiguousarray(weight, dtype=np.float32)
    bias = np.ascontiguousarray(bias, dtype=np.float32)

    # weight [k, co, ci, kd, kh, kw] -> wrev [kh, kd, ci, kw, k'(=2-k), co]
    wrev = weight.transpose(4, 3, 2, 5, 0, 1)[:, :, :, :, ::-1, :]
    # Parity block layouts (col = mb*32 + co):
    #   par=0 (even j): blocks 0..2 = k reversed, block 3 zero
    #   par=1 (odd  j): block 0 zero, blocks 1..3 = k reversed
    w_par = np.zeros((2, 3, 3, 8, 3, 4, 32), np.float32)  # [par,kh,kd,ci,kw,mb,co]
    w_par[0, :, :, :, :, 0:3] = wrev
    w_par[1, :, :, :, :, 1:4] = wrev
    # Rotated row layouts: row r = s*72 + kw*24 + kh*8 + ci, kd = (s - rot)%3
    w1h = np.zeros((R1, 12, 128), np.float32)
    w2h = np.zeros((R2, 12, 128), np.float32)
    brev = bias[::-1].reshape(96)
    for rot in range(3):
        kd_of_s = [(s - rot) % 3 for s in range(3)]
        for par in range(2):
            # [kh, s, ci, kw, mb, co] -> (s, kw, kh, ci, mb*co)
            arr = w_par[par][:, kd_of_s]
            arr = arr.transpose(1, 3, 0, 2, 4, 5).reshape(216, 128)
            a1, a2 = arr[0:128], arr[128:216]
            b2 = np.zeros(128, np.float32)
            if par == 0:
                b2[0:96] = brev
            else:
                b2[32:128] = brev
            for pair in range(2):
                g = rot * 4 + pair * 2 + par
                if pair == 0:
                    w1h[:, g] = a1
                    w2h[0:88, g] = a2
                    w2h[88, g] = b2
                else:
                    # swap column-block halves: B's o-slices t0+1/t0+2
                    # move to partitions 64:128 of the psum
                    w1h[:, g] = np.concatenate([a1[:, 64:128], a1[:, 0:64]], 1)
                    w2h[0:88, g] = np.concatenate([a2[:, 64:128], a2[:, 0:64]], 1)
                    w2h[88, g] = np.concatenate([b2[64:128], b2[0:64]])
    w1h = np.ascontiguousarray(w1h.reshape(R1, 12 * 128)).astype(bf16)
    w2h = np.ascontiguousarray(w2h.reshape(R2, 12 * 128)).astype(bf16)
    onesh = np.ones((1, FRAMES * HW), bf16)

    xb16 = x.astype(bf16)
    PL = DP * HP * WP
    in_maps = []
    for c in range(NCORES):
        b, tb = divmod(c, 4)
        t0 = tb * FRAMES
        # padded frames with 8 elems of tail slack for the window overhang
        buf = np.zeros((FRAMES, CI, PL + 8), bf16)
        xpc = buf[:, :, :PL].reshape(FRAMES, CI, DP, HP, WP)
        xpc[:, :, 1 : 1 + D, 1 : 1 + H, 1 : 1 + W] = xb16[
            b, :, t0 : t0 + FRAMES
        ].transpose(1, 0, 2, 3, 4)
        js, cs, es = buf.strides
        ds, hs, ws = HP * WP * es, WP * es, es
        # slab[plane, (kw kh ci), j, f] = flat window @ kh*42+kw of plane
        win = np.lib.stride_tricks.as_strided(
            buf,
            shape=(DP, 3, 3, CI, FRAMES, HW),
            strides=(ds, ws, hs, cs, js, ws),
        )
        slab = np.ascontiguousarray(win).reshape(DP, 72, FRAMES * HW)
        in_maps.append(
            {"xs": slab, "w1": w1h, "w2": w2h, "ones": onesh}
        )
    return in_maps


def _assemble(results):
    out = np.zeros((B, CO, O, D, H, W), np.float32)
    for c in range(NCORES):
        b, tb = divmod(c, 4)
        t0 = tb * FRAMES
        A = results[c]["outA"].astype(np.float32).reshape(4, 32, D, H, W)
        Bv = results[c]["outB"].astype(np.float32).reshape(2, 32, D, H, W)
        for i in range(4):
            o = t0 - 1 + i
            if 0 <= o < O:
                out[b, :, o] += A[i]
        for i in range(2):
            o = t0 + 3 + i
            if 0 <= o < O:
                out[b, :, o] += Bv[i]
    return out


def _run(x, weight, bias, trace=False, tmpdir=None):
    from concourse.bass_utils import run_bass_kernel_spmd

    if trace:
        _install_ntff_hook()
    nc = _get_nc()
    in_maps = _host_inputs(x, weight, bias)
    res = run_bass_kernel_spmd(
        nc,
        in_maps,
        core_ids=list(range(NCORES)),
        trace=trace,
        tmpdir=tmpdir,
    )
    return _assemble(res.results), res.exec_time_ns


def _install_ntff_hook():
    """Register the axon NTFF profile hook (missing from this image's antenv)."""
    import sys, types

    if "antenv.axon_hooks" in sys.modules:
        return
    mod = types.ModuleType("antenv.axon_hooks")
    holder = [None]
    mod.set_axon_ntff_profile_hook = lambda h: holder.__setitem__(0, h)
    mod.get_axon_ntff_profile_hook = lambda: holder[0]
    sys.modules["antenv.axon_hooks"] = mod
    try:
        from trn_agent_boot.trn_boot import _ntff_profile_via_ctypes

        mod.set_axon_ntff_profile_hook(
            _ntff_profile_via_ctypes("/opt/axon/libaxon_pjrt.so")
        )
    except Exception:
        pass


def kernel(x, weight, bias):
    out, _ = _run(x, weight, bias, trace=False)
    return out


# revision 31
# speedup vs baseline: 1.0436x; 1.0124x over previous
"""Trainium2 Bass kernel for nn_Conv4d (K separate Conv3d layers folded into a
single conv3d with K*Co output channels + temporal accumulation).

Problem (hardcoded):
  x:      [B=2, Ci=8, T=16, D=40, H=40, W=40] f32
  weight: [K=3, Co=32, Ci=8, 3, 3, 3] f32
  bias:   [K=3, Co=32] f32
  out:    [B=2, Co=32, O=16, 40, 40, 40] f32
  frame j contributes through filter k to output frame o = j + 1 - k
  (OOB temporal frames skipped, incl. their bias).

Sharding: data-parallel over the 32 B*T frames, 4 consecutive frames per
core; the remaining cross-core output-frame overlap is summed on the host.

Design (~3.4x over the v1 baseline, ~300 us vs 1019 us):
  * bf16 data / fp32 psum. rel err ~3e-3 vs the 2e-2 gate.
  * Full 216-row contraction (kw3 x kh3 x kd3 x ci8) packed into partition
    rows, 2 matmul passes per psum region (128 + 88 rows + ones/bias row)
    instead of 3 kw-shifted passes: 1280 matmuls of N=400, the tensor floor.
    Full-128-row passes also keep the PE HAM clock-gate at K=8/8 (2.4 GHz,
    ~167 ns/matmul); narrow-contraction versions of this kernel never left
    the cold 1.2 GHz state.
  * The (kw, kh) window replication (9x) is prebuilt on the host per PLANE;
    kd is realized by keeping a rolling window of 3 padded d-planes resident
    in SBUF (slot = plane % 3 -> row range) and rotating the WEIGHT rows per
    d (kd = (slot - d) mod 3; 3 precomputed weight variants), so each
    plane's replicated form is DMA'd once: input HBM ~41 MB/core.
    Slot writes are split per-frame (j0/j1 after psum-pair A whose matmuls
    were their WAR dependency, j2/j3 after pair B) in ~36-row pieces
    alternated over the gpsimd/scalar issue queues so the transfers hide
    inside the next d-slice's compute.
  * psum tile A accumulates frames j=0,1 (2 passes x 2 j per 512-col
    h-chunk region), tile B j=2,3. M = 128 = 4 blocks of 32 channels;
    parity block layouts keep every eviction a partition-base-0 op.
  * Pair B's weight column blocks are swapped (+2 mod 4) so B's overlap
    o-slices (t0+1, t0+2) land on partitions 64:128; after evicting both
    pairs to bf16 SBUF stages (scalar engine for A, vector for B), a single
    aligned vector add merges B's overlap into stage A and only B's unique
    half ships: output HBM 24.6 MB/core instead of 32.8.
  * Output DMAs go as partition halves on the sync queue; issue queues are
    kept single-purpose (sync: outputs+wt1, gpsimd/scalar: slot pieces,
    alternating) -- a dma_start's semaphore wait blocks its whole queue, so
    mixing late-resolving slot writes with output DMAs stalls both.
Host: final fp32 output assembled from bf16 partials (A rows -> o = t0-1+i
fully merged for the in-core overlap; B rows -> o = t0+3+i; cross-core
boundary frames summed across cores).
"""

import numpy as np

_STATE: dict = {}

# ---- problem constants --------------------------------------------------
B, CI, T, D, H, W = 2, 8, 16, 40, 40, 40
K, CO = 3, 32
O = 16
NCORES = 8
FRAMES = 4          # frames per core
DP, HP, WP = D + 2, H + 2, W + 2   # padded dims
HW = H * WP          # 40*42 = free size of one (h,w') window
NHC = 4              # h-chunks per d-slice
HCROWS = H // NHC    # 10 rows -> N=400 per matmul
R1 = 128             # pass-1 contraction rows
R2 = 89              # pass-2 rows (88 data + ones)


def _build_nc():
    import concourse.mybir as mybir
    from concourse import bacc
    from concourse.tile import TileContext

    f32 = mybir.dt.float32
    bf16 = mybir.dt.bfloat16

    nc = bacc.Bacc(
        "TRN2", target_bir_lowering=False, debug=False, num_devices=NCORES
    )
    # xslab[plane, row(kw kh ci), j*window]
    xs = nc.dram_tensor("xs", [DP, 72, FRAMES * HW], bf16, kind="ExternalInput")
    w1 = nc.dram_tensor("w1", [R1, 12 * 128], bf16, kind="ExternalInput")
    w2 = nc.dram_tensor("w2", [R2, 12 * 128], bf16, kind="ExternalInput")
    ones = nc.dram_tensor("ones", [1, FRAMES * HW], bf16, kind="ExternalInput")
    outA = nc.dram_tensor("outA", [128, D * H * W], bf16, kind="ExternalOutput")
    outB = nc.dram_tensor("outB", [64, D * H * W], bf16, kind="ExternalOutput")

    FHW = FRAMES * HW
    QS = None  # set below

    with TileContext(nc) as tc:
        with (
            tc.tile_pool(name="const", bufs=1) as pc,
            tc.tile_pool(name="st", bufs=4) as pst,
            tc.tile_pool(name="ps", bufs=2, space="PSUM") as pp,
        ):
            wt1 = pc.tile([R1, 12 * 128], bf16)
            wt2 = pc.tile([R2, 12 * 128], bf16)
            xa = pc.tile([R1, FHW], bf16)
            xb = pc.tile([R2, FHW], bf16)
            # dependency-free dummy tile: warm-up matmuls on it can issue
            # immediately, releasing the PE HAM clock gate (1.2 -> 2.4 GHz)
            # while the startup DMAs fill SBUF
            dm = pc.tile([128, 512], bf16)
            nc.gpsimd.memset(dm[:, :], 0)
            wps = pp.tile([128, NHC * 512], f32, tag="ps")
            for _ in range(24):
                nc.tensor.matmul(
                    wps[:, 0:512], dm[:, 0:128], dm[:, :],
                    start=True, stop=True, skip_group_check=True,
                )
            # weight columns in need-order: d=0 uses cols 0:512 (rot 0),
            # d=1 cols 512:1024, d=2 cols 1024:1536
            nc.scalar.dma_start(xb[88:89, :], ones[:, :])
            nc.sync.dma_start(wt1[:, 0:512], w1[:, 0:512])
            nc.sync.dma_start(wt2[:, 0:512], w2[:, 0:512])
            QS = [nc.sync, nc.gpsimd, nc.scalar]

            def write_chunk(plane, jc, qs=None):
                """DMA one j-chunk of plane's 72 replicated rows to its slot.

                Split into ~24-row pieces on alternating queues: each
                dma_start is one descriptor; per-DMA-engine bandwidth is only
                ~19 GB/s regardless of packet size, so throughput and arrival
                latency come from many outstanding descriptors fanning out
                across the 16 engines. Slot chunks' semaphore waits resolve
                late, so keep them off the sync queue (which carries the
                output DMAs) to avoid head-of-line blocking.
                """
                qs = qs or [nc.gpsimd, nc.scalar]
                s = plane % 3
                lo = jc * HW
                hi = lo + HW
                if s == 0:
                    pieces = [(xa, 0, 0, 36), (xa, 36, 36, 72)]
                elif s == 1:
                    pieces = [(xa, 72, 0, 28), (xa, 100, 28, 56), (xb, 0, 56, 72)]
                else:
                    pieces = [(xb, 16, 0, 36), (xb, 52, 36, 72)]
                for i, (tile, db, sa, sb) in enumerate(pieces):
                    qs[i % len(qs)].dma_start(
                        tile[db : db + sb - sa, lo:hi], xs[plane, sa:sb, lo:hi]
                    )

            # initial planes 0-2: j=0 chunks of all slots first (the first
            # matmuls need j=0 rows of every slot), spread over 3 queues;
            # later-needed weight columns interleave after the j0/j1 waves
            for jc in range(FRAMES):
                for p in range(3):
                    r = p + jc
                    write_chunk(p, jc, [QS[r % 3], QS[(r + 1) % 3]])
                if jc == 0:
                    nc.sync.dma_start(wt1[:, 512:1024], w1[:, 512:1024])
                    nc.gpsimd.dma_start(wt2[:, 512:1024], w2[:, 512:1024])
                elif jc == 1:
                    nc.sync.dma_start(wt1[:, 1024:1536], w1[:, 1024:1536])
                    nc.gpsimd.dma_start(wt2[:, 1024:1536], w2[:, 1024:1536])

            xav = xa[:, :].rearrange("p (j h w) -> p j h w", j=FRAMES, w=WP)
            xbv = xb[:, :].rearrange("p (j h w) -> p j h w", j=FRAMES, w=WP)
            for d in range(D):
                rot = d % 3
                for pair in range(2):
                    ps = pp.tile([128, NHC * 512], f32, tag="ps")
                    for jj in range(2):
                        j = pair * 2 + jj
                        g = rot * 4 + pair * 2 + jj
                        l1 = wt1[:, g * 128 : (g + 1) * 128]
                        l2 = wt2[:, g * 128 : (g + 1) * 128]
                        for hc in range(NHC):
                            out_ap = ps[:, hc * 512 : hc * 512 + HCROWS * W]
                            rhs1 = xav[
                                :, j, hc * HCROWS : (hc + 1) * HCROWS, 0:W
                            ]
                            rhs2 = xbv[
                                :, j, hc * HCROWS : (hc + 1) * HCROWS, 0:W
                            ]
                            nc.tensor.matmul(
                                out_ap, l1, rhs1,
                                start=(jj == 0), stop=False,
                            )
                            nc.tensor.matmul(
                                out_ap, l2, rhs2,
                                start=False, stop=(jj == 1),
                            )
                    psv = ps[:, :].rearrange("p (b c) -> p b c", c=512)[
                        :, :, 0 : HCROWS * W
                    ]
                    lo, hi = d * H * W, (d + 1) * H * W
                    if pair == 0:
                        stA = pst.tile([128, H * W], bf16, tag="stA")
                        sv = stA[:, :].rearrange("p (b c) -> p b c", c=HCROWS * W)
                        nc.scalar.copy(sv, psv)
                        nc.sync.dma_start(outA[0:64, lo:hi], stA[0:64, :])
                    else:
                        # pair B's weight blocks are swapped (+2 mod 4) so its
                        # overlap o-slices (t0+1, t0+2) land at partitions
                        # 64:128: merge them into stA on-chip, ship only
                        # B's unique half -- output traffic 32.8 -> 24.6 MB
                        stB = pst.tile([128, H * W], bf16, tag="stB")
                        sv = stB[:, :].rearrange("p (b c) -> p b c", c=HCROWS * W)
                        nc.vector.tensor_copy(sv, psv)
                        nc.vector.tensor_add(
                            stA[64:128, :], stB[64:128, :], stA[64:128, :]
                        )
                        nc.sync.dma_start(outA[64:128, lo:hi], stA[64:128, :])
                        nc.sync.dma_start(outB[0:64, lo:hi], stB[0:64, :])
                    # prefetch plane d+3: chunks j0/j1 right after pair A
                    # (their WAR deps -- readers of plane d, frames 0/1 --
                    # are already satisfied), j2/j3 after pair B
                    if d + 3 < DP:
                        for jc in (0, 1) if pair == 0 else (2, 3):
                            write_chunk(d + 3, jc)
    nc.compile()
    return nc


def _get_nc():
    if "nc" not in _STATE:
        _STATE["nc"] = _build_nc()
    return _STATE["nc"]


def _host_inputs(x, weight, bias):
    """Build per-core input maps."""
    import ml_dtypes

    bf16 = ml_dtypes.bfloat16
    x = np.asarray(x, dtype=np.float32)
    weight = np.ascontiguousarray(weight, dtype=np.float32)
    bias = np.ascontiguousarray(bias, dtype=np.float32)

    # weight [k, co, ci, kd, kh, kw] -> wrev [kh, kd, ci, kw, k'(=2-k), co]
    wrev = weight.transpose(4, 3, 2, 5, 0, 1)[:, :, :, :, ::-1, :]
    # Parity block layouts (col = mb*32 + co):
    #   par=0 (even j): blocks 0..2 = k reversed, block 3 zero
    #   par=1 (odd  j): block 0 zero, blocks 1..3 = k reversed
    w_par = np.zeros((2, 3, 3, 8, 3, 4, 32), np.float32)  # [par,kh,kd,ci,kw,mb,co]
    w_par[0, :, :, :, :, 0:3] = wrev
    w_par[1, :, :, :, :, 1:4] = wrev
    # Rotated row layouts: row r = s*72 + kw*24 + kh*8 + ci, kd = (s - rot)%3
    w1h = np.zeros((R1, 12, 128), np.float32)
    w2h = np.zeros((R2, 12, 128), np.float32)
    brev = bias[::-1].reshape(96)
    for rot in range(3):
        kd_of_s = [(s - rot) % 3 for s in range(3)]
        for par in range(2):
            # [kh, s, ci, kw, mb, co] -> (s, kw, kh, ci, mb*co)
            arr = w_par[par][:, kd_of_s]
            arr = arr.transpose(1, 3, 0, 2, 4, 5).reshape(216, 128)
            a1, a2 = arr[0:128], arr[128:216]
            b2 = np.zeros(128, np.float32)
            if par == 0:
                b2[0:96] = brev
            else:
                b2[32:128] = brev
            for pair in range(2):
                g = rot * 4 + pair * 2 + par
                if pair == 0:
                    w1h[:, g] = a1
                    w2h[0:88, g] = a2
                    w2h[88, g] = b2
                else:
                    # swap column-block halves: B's o-slices t0+1/t0+2
                    # move to partitions 64:128 of the psum
                    w1h[:, g] = np.concatenate([a1[:, 64:128], a1[:, 0:64]], 1)
                    w2h[0:88, g] = np.concatenate([a2[:, 64:128], a2[:, 0:64]], 1)
                    w2h[88, g] = np.concatenate([b2[64:128], b2[0:64]])
    w1h = np.ascontiguousarray(w1h.reshape(R1, 12 * 128)).astype(bf16)
    w2h = np.ascontiguousarray(w2h.reshape(R2, 12 * 128)).astype(bf16)
    onesh = np.ones((1, FRAMES * HW), bf16)

    xb16 = x.astype(bf16)
    PL = DP * HP * WP
    in_maps = []
    for c in range(NCORES):
        b, tb = divmod(c, 4)
        t0 = tb * FRAMES
        # padded frames with 8 elems of tail slack for the window overhang
        buf = np.zeros((FRAMES, CI, PL + 8), bf16)
        xpc = buf[:, :, :PL].reshape(FRAMES, CI, DP, HP, WP)
        xpc[:, :, 1 : 1 + D, 1 : 1 + H, 1 : 1 + W] = xb16[
            b, :, t0 : t0 + FRAMES
        ].transpose(1, 0, 2, 3, 4)
        js, cs, es = buf.strides
        ds, hs, ws = HP * WP * es, WP * es, es
        # slab[plane, (kw kh ci), j, f] = flat window @ kh*42+kw of plane
        win = np.lib.stride_tricks.as_strided(
            buf,
            shape=(DP, 3, 3, CI, FRAMES, HW),
            strides=(ds, ws, hs, cs, js, ws),
        )
        slab = np.ascontiguousarray(win).reshape(DP, 72, FRAMES * HW)
        in_maps.append(
            {"xs": slab, "w1": w1h, "w2": w2h, "ones": onesh}
        )
    return in_maps


def _assemble(results):
    out = np.zeros((B, CO, O, D, H, W), np.float32)
    for c in range(NCORES):
        b, tb = divmod(c, 4)
        t0 = tb * FRAMES
        A = results[c]["outA"].astype(np.float32).reshape(4, 32, D, H, W)
        Bv = results[c]["outB"].astype(np.float32).reshape(2, 32, D, H, W)
        for i in range(4):
            o = t0 - 1 + i
            if 0 <= o < O:
                out[b, :, o] += A[i]
        for i in range(2):
            o = t0 + 3 + i
            if 0 <= o < O:
                out[b, :, o] += Bv[i]
    return out


def _run(x, weight, bias, trace=False, tmpdir=None):
    from concourse.bass_utils import run_bass_kernel_spmd

    if trace:
        _install_ntff_hook()
    nc = _get_nc()
    in_maps = _host_inputs(x, weight, bias)
    res = run_bass_kernel_spmd(
        nc,
        in_maps,
        core_ids=list(range(NCORES)),
        trace=trace,
        tmpdir=tmpdir,
    )
    return _assemble(res.results), res.exec_time_ns


def _install_ntff_hook():
    """Register the axon NTFF profile hook (missing from this image's antenv)."""
    import sys, types

    if "antenv.axon_hooks" in sys.modules:
        return
    mod = types.ModuleType("antenv.axon_hooks")
    holder = [None]
    mod.set_axon_ntff_profile_hook = lambda h: holder.__setitem__(0, h)
    mod.get_axon_ntff_profile_hook = lambda: holder[0]
    sys.modules["antenv.axon_hooks"] = mod
    try:
        from trn_agent_boot.trn_boot import _ntff_profile_via_ctypes

        mod.set_axon_ntff_profile_hook(
            _ntff_profile_via_ctypes("/opt/axon/libaxon_pjrt.so")
        )
    except Exception:
        pass


def kernel(x, weight, bias):
    out, _ = _run(x, weight, bias, trace=False)
    return out
